# revision 26
# baseline (speedup 1.0000x reference)
"""Trainium2 Bass kernel for nn_AttentionModel (sparse banded attention).

Math (per batch element, data-parallel over 8 cores):
  scores = q @ (k @ W_score)^T          # W_score folded into k on host
  w      = banded_softmax(scores)       # full-row max cancels mathematically
  c      = w @ k
  enh    = tanh(concat([c, q]) @ W_enh.T + b_enh)
  out    = sigmoid(enh @ W_mask.T + b_mask)

Implementation notes (v3):
  - W_score is folded into the keys on host (kp = k @ W_score) so there is
    no on-chip qs phase; scores use qT directly as the stationary operand.
  - T=2000 padded: keys 128 front + 48 tail -> 2176 = 17*128; queries 48
    tail -> 2048 = 16*128.  Query tile j attends key blocks j, j+1.
  - Score PSUM banks hold two adjacent tiles; ONE exp per pair reads the
    raw PSUM scores (no mask pass).  The band mask is a 0/1 multiply fused
    with the row-sum reduce on the DVE (tensor_tensor_reduce), so the ACT
    accumulator readout disappears.
  - w is normalized and cast to fp8 in one DVE op; transposes run on the
    PE in fp8; PV is one fp8 DoubleRow matmul per tile per h-chunk
    (key blocks j, j+1 interleaved), with kN stored in fp8.
  - P2: q-half bf16 exact; c-half fp8 DoubleRow.  P2 weights pre-scaled
    x16 on host (fp8 weights stay out of the subnormal range); ACT tanh
    applies scale=1/16.
  - P3 bf16; the b_mask add runs on gpsimd (reads PSUM), tanh on ACT,
    final 0.5x+0.5 on gpsimd; sigmoid(x) = 0.5*tanh(0.5x)+0.5 keeps ACT
    on one table set (exp+tanh).
  - Tail: P2 for tiles 12..15 is split (2/1/1 tiles) so only p3(15)
    remains after the loop.
  - DMA: interleaved host layouts give few, large descriptors; loads are
    ordered by first need (sync: kp + q-mid + P2/P3 consts + stores;
    scalar: q-head + small consts only, to protect the ACT engine;
    gpsimd: kN + q-tail).
"""

import sys
import types

import numpy as np
import ml_dtypes
from contextlib import ExitStack

import concourse.bass as bass
import concourse.bacc as bacc
import concourse.tile as tile
from concourse import mybir
from concourse.bass_utils import run_bass_kernel_spmd


def _ensure_axon_hooks():
    try:
        from antenv import axon_hooks  # noqa: F401
        return
    except ImportError:
        pass
    try:
        from trn_agent_boot.trn_boot import _ntff_profile_via_ctypes
        hook = _ntff_profile_via_ctypes("/opt/axon/libaxon_pjrt.so")
    except Exception:
        hook = None
    m = types.ModuleType("antenv.axon_hooks")
    m.get_axon_ntff_profile_hook = lambda: hook
    m.set_axon_ntff_profile_hook = lambda h: None
    sys.modules["antenv.axon_hooks"] = m


_ensure_axon_hooks()

F32 = mybir.dt.float32
BF16 = mybir.dt.bfloat16
FP8 = mybir.dt.float8e4
AF = mybir.ActivationFunctionType
ALU = mybir.AluOpType
DRM = mybir.MatmulPerfMode.DoubleRow

NP_BF16 = ml_dtypes.bfloat16
NP_FP8 = ml_dtypes.float8_e4m3

B, T, H, F_OUT = 8, 2000, 256, 257
TPK = 2176   # padded key length   (128 front + 2000 + 48 tail)
TPQ = 2048   # padded query length (2000 + 48 tail)
NT = 16      # query tiles of 128
NKB = 17     # key blocks of 128
OPAD = 258   # F_OUT padded even
N_CORES = 8
WSC = 16.0   # fp8 / P2 weight pre-scale

_CACHE = {}


def _consts():
    t_i = np.arange(128, dtype=np.int32)[:, None]
    s_i = np.arange(128, dtype=np.int32)[None, :]
    b_prev = (s_i >= t_i).astype(np.float32)
    b_diag = (s_i <= t_i).astype(np.float32)
    band_std = np.concatenate([b_prev, b_diag], 1)
    band_t0 = np.concatenate([np.zeros((128, 128), np.float32), b_diag], 1)
    return np.ascontiguousarray(
        np.concatenate([band_std, band_t0], 1).astype(NP_BF16))


def build_nc():
    nc = bacc.Bacc("TRN2", target_bir_lowering=False, debug=False,
                   num_devices=N_CORES)

    kpT = nc.declare_dram_parameter("kpT", [128, 2 * TPK], BF16, isOutput=False)
    qT = nc.declare_dram_parameter("qT", [128, 2 * TPQ], BF16, isOutput=False)
    kN8 = nc.declare_dram_parameter("kN8", [128, NKB * 256], FP8,
                                    isOutput=False)
    WeqT = nc.declare_dram_parameter("WeqT", [128, 2 * H], BF16, isOutput=False)
    Wec8 = nc.declare_dram_parameter("Wec8", [128, 2 * H], FP8, isOutput=False)
    WmT16 = nc.declare_dram_parameter("WmT16", [128, 2 * OPAD], BF16,
                                      isOutput=False)
    be = nc.declare_dram_parameter("be", [128, 2], F32, isOutput=False)
    bm128 = nc.declare_dram_parameter("bm128", [128, OPAD], BF16,
                                      isOutput=False)
    out = nc.declare_dram_parameter("out", [T, F_OUT], F32, isOutput=True)

    band_d = nc.inline_tensor(_consts().view(np.uint16), "bandc")
    ident8_np = (np.eye(128, dtype=np.uint16) * 0x3F80).astype(np.uint16)
    ident8_d = nc.inline_tensor(ident8_np, "identc")

    with tile.TileContext(nc) as tc, ExitStack() as ctx:
        const = ctx.enter_context(tc.tile_pool(name="const", bufs=1))
        io = ctx.enter_context(tc.tile_pool(name="io", bufs=1))
        wk = ctx.enter_context(tc.tile_pool(name="wk", bufs=4))
        stat = ctx.enter_context(tc.tile_pool(name="stat", bufs=8))
        pmm = ctx.enter_context(tc.tile_pool(name="pmm", bufs=2, space="PSUM"))
        psc = ctx.enter_context(tc.tile_pool(name="psc", bufs=4, space="PSUM"))
        pct = ctx.enter_context(tc.tile_pool(name="pct", bufs=1, space="PSUM"))
        pwt = ctx.enter_context(tc.tile_pool(name="pwt", bufs=1, space="PSUM"))

        # ---- SBUF tiles ----
        qT_t = io.tile([128, 2 * TPQ], BF16, tag="qT", name="qT_t")
        kpT_t = io.tile([128, 2 * TPK], BF16, tag="kpT", name="kpT_t")
        kN_t = io.tile([128, NKB * 256], FP8, tag="kN", name="kN_t")
        c8_t = io.tile([128, 2 * TPQ], FP8, tag="c8", name="c8_t")
        enh_t = io.tile([128, 2 * TPQ], BF16, tag="enh", name="enh_t")
        wTall = io.tile([128, NT * 256], FP8, tag="wTall", name="wTall")

        qv = qT_t[:].rearrange("p (c x) -> p c x", x=TPQ)        # [128,2,2048]
        kpv = kpT_t[:].rearrange("p (c x) -> p c x", x=TPK)      # [128,2,2176]
        kNv = kN_t[:].rearrange("p (b x) -> p b x", x=256)       # [128,17,256]
        c8v = c8_t[:].rearrange("p (i x) -> p i x", x=TPQ)       # [128,2,2048]
        env = enh_t[:].rearrange("p (i x) -> p i x", x=TPQ)      # [128,2,2048]

        def cload(tag, shape, src, dt, q=nc.sync):
            t = const.tile(shape, dt, tag=tag, name=tag)
            q.dma_start(t[:], src)
            return t

        # ---- DMA: ordered by first need across three rings ----
        def load_q(a, b, q=nc.sync):
            for c in range(2):
                q.dma_start(qT_t[:, c * TPQ + a: c * TPQ + b],
                            qT[:, c * TPQ + a: c * TPQ + b])

        def load_kp(a, b, q=nc.sync):
            for c in range(2):
                q.dma_start(kpT_t[:, c * TPK + a: c * TPK + b],
                            kpT[:, c * TPK + a: c * TPK + b])

        def load_kn(b0, b1):
            nc.gpsimd.dma_start(kN_t[:, b0 * 256: b1 * 256],
                                kN8[:, b0 * 256: b1 * 256])

        load_kp(0, 256)
        load_q(0, 512, q=nc.scalar)
        load_kn(0, 4)
        band_t = cload("band", [128, 512], band_d[:], mybir.dt.uint16,
                       q=nc.scalar)
        band = band_t[:].bitcast(BF16)
        ident8_t = cload("ident", [128, 128], ident8_d[:], mybir.dt.uint16,
                         q=nc.scalar)
        ident = ident8_t[:].bitcast(BF16)
        load_kp(256, 512)
        load_kp(512, 1024)
        weq = cload("weq", [128, 2 * H], WeqT[:], BF16)
        wec8 = cload("wec8", [128, 2 * H], Wec8[:], FP8)
        bet = cload("bet", [128, 2], be[:], F32)
        load_kn(4, 8)
        load_q(512, 1024, q=nc.gpsimd)
        load_kp(1024, 1536)
        wmt = cload("wmt", [128, 2 * OPAD], WmT16[:], BF16)
        bm_t = cload("bm", [128, OPAD], bm128[:], BF16)
        load_kn(8, 12)
        load_q(1024, 1536, q=nc.gpsimd)
        load_kp(1536, 2176)
        load_kn(12, 17)
        load_q(1536, 2048, q=nc.gpsimd)

        weqv = weq[:].rearrange("p (d f) -> p d f", f=H)         # [128,2,256]
        wecv = wec8[:].rearrange("p (i f) -> p i f", f=H)        # [128,2,256]
        wmv = wmt[:].rearrange("p (f o) -> p f o", o=OPAD)       # [128,2,258]

        # ---- per-tile attention stages ----
        score_bank = {}

        def scores(t):
            score_bank[t] = psc.tile([128, 256], F32, tag="sc", name="ps")
            ps = score_bank[t][:]
            for c in range(2):
                nc.tensor.matmul(
                    ps,
                    qv[:, c, t * 128:(t + 1) * 128],
                    kpv[:, c, t * 128: t * 128 + 256],
                    start=(c == 0), stop=(c == 1))
            return ps

        e_pair = {}

        def exp_pair(p):
            pass

        def softmax(j):
            # exp per tile, straight from PSUM
            e2 = wk.tile([128, 256], BF16, tag="e2", name="e2")
            nc.scalar.activation(e2[:], score_bank.pop(j)[:], AF.Exp)
            eh = e2[:]
            # band mask (0/1 multiply) fused with the row-sum reduce
            wu = wk.tile([128, 256], BF16, tag="wu", name="wu")
            den = stat.tile([128, 1], F32, tag="den", name="den")
            boff = 256 if j == 0 else 0
            nc.vector.tensor_mul(wu[:], eh, band[:, boff:boff + 256])
            nc.vector.tensor_reduce(den[:], wu[:], mybir.AxisListType.X,
                                    ALU.add)
            rec = stat.tile([128, 1], F32, tag="rec", name="rec")
            nc.vector.reciprocal(rec[:], den[:])
            w_t = wk.tile([128, 256], BF16, tag="w8", name="w8")
            nc.vector.tensor_scalar_mul(w_t[:], wu[:], rec[:])
            pw = pwt.tile([128, 256], BF16, tag="pw", name="pw")
            nc.tensor.transpose(pw[:, 0:128], w_t[:, 0:128], ident)
            nc.tensor.transpose(pw[:, 128:256], w_t[:, 128:256], ident)
            nc.vector.tensor_copy(wTall[:, j * 256:(j + 1) * 256], pw[:])

        def pvdr(j):
            # PV: one fp8 DoubleRow matmul per h-chunk (key blocks j, j+1)
            pc = pct.tile([128, 256], F32, tag="pc", name="pc")
            wT = wTall[:, j * 256:(j + 1) * 256].rearrange(
                "p (b t) -> p b t", t=128)
            for h in range(2):
                for b in range(2):
                    nc.tensor.matmul(
                        pc[:, h * 128:(h + 1) * 128],
                        kNv[:, j + b, h * 128:(h + 1) * 128],
                        wT[:, b, :],
                        start=(b == 0), stop=(b == 1),
                        skip_group_check=(h == 1))
            nc.vector.tensor_copy(
                c8v[:, 0:2, j * 128:(j + 1) * 128],
                pc[:].rearrange("p (b x) -> p b x", x=128))

        def p2(t0, tw):
            # enhT[f, t'] = tanh((16*W_enh).T @ [cT; qT] / 16 + b_enh)
            for f in range(2):
                pe_ = pmm.tile([128, tw], F32, tag="mm", name="pe_")
                for d in range(2):
                    nc.tensor.matmul(
                        pe_[:],
                        weqv[:, d, f * 128:(f + 1) * 128],
                        qv[:, d, t0:t0 + tw],
                        start=(d == 0), stop=False)
                nc.tensor.matmul(
                    pe_[:],
                    wecv[:, 0:2, f * 128:(f + 1) * 128],
                    c8v[:, 0:2, t0:t0 + tw],
                    start=False, stop=True, perf_mode=DRM)
                nc.scalar.activation(
                    env[:, f:f + 1, t0:t0 + tw],
                    pe_[:].rearrange("p (b x) -> p b x", x=tw),
                    AF.Tanh, scale=1.0 / WSC, bias=bet[:, f:f + 1])

        def p3(j):
            # z = enh @ W_mask.T + b_mask ; out = 0.5*tanh(z/2)+0.5
            pm = pmm.tile([128, OPAD], F32, tag="mm", name="pm")
            for f in range(2):
                nc.tensor.matmul(
                    pm[:],
                    env[:, f:f + 1, j * 128:(j + 1) * 128],
                    wmv[:, f, :],
                    start=(f == 0), stop=(f == 1))
            z_t = wk.tile([128, OPAD], F32, tag="z", name="z_t")
            nc.vector.tensor_add(z_t[:], pm[:], bm_t[:])
            o_t = wk.tile([128, OPAD], F32, tag="o", name="o_t")
            nc.scalar.activation(o_t[:], z_t[:], AF.Tanh, scale=0.5)
            p = j // 2
            if j % 2 == 0:
                _CACHE[f"o2pair{p}"] = wk.tile([128, 2 * OPAD], F32, tag="o2",
                                               name="o2_t")
            o2_t = _CACHE.pop(f"o2pair{p}") if j % 2 else _CACHE[f"o2pair{p}"]
            half = o2_t[:, (j % 2) * OPAD:(j % 2 + 1) * OPAD]
            nc.gpsimd.tensor_scalar(half, o_t[:], 0.5, 0.5,
                                    op0=ALU.mult, op1=ALU.add)
            if p == 7 and j % 2 == 0:
                nc.sync.dma_start(out[1792:1920, :], o2_t[:, 0:F_OUT])
            if j % 2 == 1:
                src_v = o2_t[:].rearrange("p (b o) -> p b o", o=OPAD)
                if p < 7:
                    nc.sync.dma_start(
                        out[p * 256:(p + 1) * 256, :].rearrange(
                            "(b p2) o -> p2 b o", p2=128),
                        src_v[:, :, 0:F_OUT])
                else:
                    nc.sync.dma_start(out[1920:2000, :],
                                      o2_t[0:80, OPAD:OPAD + F_OUT])

        # ---- attention loop, software-pipelined ----
        for jj in range(4):
            scores(jj)
        pending_p3 = []
        for j in range(NT):
            if j % 2 == 0:
                exp_pair(j // 2)
            softmax(j)
            if j + 4 < NT:
                scores(j + 4)
            if j in (4, 8, 12):
                p2((j - 4) * 128, 512)
                pending_p3.extend(range(j - 4, j))
            if j == 14:
                p2(12 * 128, 256)              # tiles 12, 13
                pending_p3.extend([12, 13])
            if j == 15:
                p2(14 * 128, 128)              # tile 14
                pending_p3.append(14)
            for _ in range(2 if j >= 13 else 1):
                if pending_p3:
                    p3(pending_p3.pop(0))
            pvdr(j)
        p2(15 * 128, 128)                      # tile 15
        p3(15)

    return nc


def _prep_shared(W_score, W_enh, b_enh, W_mask, b_mask):
    We = np.ascontiguousarray(W_enh.T.astype(np.float32))           # [d, f]
    WeqT = np.ascontiguousarray(
        (WSC * We[H:]).reshape(2, 128, H).transpose(1, 0, 2).reshape(128, 2 * H)
    ).astype(NP_BF16)
    Wec8 = np.ascontiguousarray(
        (WSC * We[:H]).reshape(2, 128, H).transpose(1, 0, 2).reshape(128, 2 * H)
    ).astype(NP_FP8)
    Wm = np.zeros((H, OPAD), np.float32)                            # [f, o]
    Wm[:, :F_OUT] = W_mask.T.astype(np.float32)
    WmT16 = np.ascontiguousarray(
        Wm.reshape(2, 128, OPAD).transpose(1, 0, 2).reshape(128, 2 * OPAD)
    ).astype(NP_BF16)
    be = np.ascontiguousarray(
        b_enh.astype(np.float32).reshape(2, 128).T)                 # [128, 2]
    bm = np.zeros((1, OPAD), np.float32)
    bm[0, :F_OUT] = b_mask.astype(np.float32)
    bm128 = np.ascontiguousarray(np.repeat(bm, 128, 0)).astype(NP_BF16)
    return WeqT, Wec8, WmT16, be, bm128


def make_in_maps(k, q, W_score, W_enh, b_enh, W_mask, b_mask):
    k = np.asarray(k, np.float32)
    q = np.asarray(q, np.float32)
    Ws = np.asarray(W_score, np.float32)
    WeqT, Wec8, WmT16, be, bm128 = _prep_shared(
        Ws, np.asarray(W_enh, np.float32), np.asarray(b_enh, np.float32),
        np.asarray(W_mask, np.float32), np.asarray(b_mask, np.float32))
    kp = k @ Ws[None]          # [B, T, H]: scores = q @ kp^T
    in_maps = []
    for b in range(N_CORES):
        kpb = np.zeros((TPK, H), np.float32)
        kpb[128:128 + T] = kp[b]
        kb = np.zeros((TPK, H), np.float32)
        kb[128:128 + T] = k[b]
        qb = np.zeros((TPQ, H), np.float32)
        qb[:T] = q[b]
        kpT = np.ascontiguousarray(
            kpb.T.reshape(2, 128, TPK).transpose(1, 0, 2).reshape(128, 2 * TPK)
        ).astype(NP_BF16)
        qT = np.ascontiguousarray(
            qb.T.reshape(2, 128, TPQ).transpose(1, 0, 2).reshape(128, 2 * TPQ)
        ).astype(NP_BF16)
        kN8 = np.ascontiguousarray(
            kb.reshape(NKB, 128, H).transpose(1, 0, 2).reshape(128, NKB * 256)
        ).astype(NP_FP8)
        in_maps.append({
            "kpT": kpT, "qT": qT, "kN8": kN8,
            "WeqT": WeqT, "Wec8": Wec8, "WmT16": WmT16,
            "be": be, "bm128": bm128,
        })
    return in_maps


def get_nc():
    if "nc" not in _CACHE:
        nc = build_nc()
        nc.finalize()
        _CACHE["nc"] = nc
    return _CACHE["nc"]


def kernel(k, q, W_score, W_enh, b_enh, W_mask, b_mask):
    in_maps = make_in_maps(k, q, W_score, W_enh, b_enh, W_mask, b_mask)
    res = run_bass_kernel_spmd(get_nc(), in_maps, list(range(N_CORES)))
    return np.stack([r["out"] for r in res.results], 0)
